# revision 15
# baseline (speedup 1.0000x reference)
"""Trainium2 Bass kernel for masked-GRU + residual + LayerNorm.

Problem: N=128 sequences of length L=512, hidden H=512.
  gx = x @ W_ih.T + b_ih            (precomputable input projection)
  per step l: hc = h * (1-is_initial[l]); gh = hc @ W_hh.T + b_hh
    r = sig(gx_r+gh_r); z = sig(gx_z+gh_z); n = tanh(gx_n + r*gh_n)
    h = (1-z)*n + z*hc
  out = LayerNorm(seq + x) * gamma + beta;  h_exp = broadcast(h_last)

Strategy:
  * Data parallel: 16 batch rows per core (8 cores).
  * Sequence-chunk parallel: each L=512 sequence is split into C=16
    chunks of 32 steps, processed as independent columns, made exact by
    an R-step warm-up (resets bound the dependency horizon).
  * Weights staged bf16 (stationary operand) -> compiler enables fast
    weight load; moving operands (state/x) stay f32r.  LDWEIGHTS at f32
    (~200ns each, 96/step) was the whole baseline bottleneck.
  * Per-step emission order keeps the ACT queue chain-first (sigmoid/
    tanh before next-step PSUM drains) so the recurrent chain finishes
    inside the PE's independent prefill window -> no PE idle, HAM stays
    at K=8/8 (2.4 GHz).
  * LayerNorm for a finished 4-step block is emitted one block later
    (inputs long-ready), stats math done at 128 partitions after a
    DRAM-bounce broadcast of the row sums.
"""
import sys

sys.path.insert(0, "/opt/trn_rl_repo")

import numpy as np
import ml_dtypes

import concourse.bass as bass
import concourse.tile as tile
from concourse import bacc, mybir
from concourse.bass_utils import run_bass_kernel_spmd

F32 = mybir.dt.float32
F32R = mybir.dt.float32r
BF16 = mybir.dt.bfloat16
AF = mybir.ActivationFunctionType
ALU = mybir.AluOpType

N, L, H = 128, 512, 512
NCORES = 8
NB = N // NCORES          # batch rows per core = 16
C = 16                    # chunks per sequence
KS = L // C               # main steps per chunk = 32
S = NB * C                # columns per core = 256
HT = H // 128             # h partition tiles = 4
BLK = 4                   # LN block (main steps)
NBLK = KS // BLK          # 8


def _bcast_ap(row_ap, parts=128):
    """DRAM row AP -> partition-broadcast AP (step 0 over partitions)."""
    return bass.AP(
        tensor=row_ap.tensor,
        offset=row_ap.offset,
        ap=[[0, parts]] + [list(d) for d in row_ap.ap],
    )


def build_program(R=12, triv_gb=False):
    T = R + KS
    nc = bacc.Bacc("TRN2", target_bir_lowering=False)

    xs_d = nc.declare_dram_parameter("xs", [HT, 128, T, S], BF16, isOutput=False)
    xr_d = nc.declare_dram_parameter("xres", [HT, 128, KS, S], F32, isOutput=False)
    ms_d = nc.declare_dram_parameter("ms", [T, S], F32, isOutput=False)
    h0m_d = nc.declare_dram_parameter("h0m", [HT, 128, NB], F32R, isOutput=False)
    wih_d = nc.declare_dram_parameter("wih", [HT, 128, 3 * H], BF16, isOutput=False)
    whh_d = nc.declare_dram_parameter("whh", [HT, 128, 3 * H], BF16, isOutput=False)
    brz_d = nc.declare_dram_parameter("brz", [128, 8], F32, isOutput=False)
    bhn_d = nc.declare_dram_parameter("bhn", [128, HT], F32, isOutput=False)
    bin_d = nc.declare_dram_parameter("bin", [128, HT], F32, isOutput=False)
    gam_d = nc.declare_dram_parameter("gam", [128, HT], F32, isOutput=False)
    bet_d = nc.declare_dram_parameter("bet", [128, HT], F32, isOutput=False)
    ones_d = nc.declare_dram_parameter("ones", [128, 1], F32R, isOutput=False)

    out_d = nc.declare_dram_parameter("out_st", [HT, 128, KS, S], F32, isOutput=True)
    hl_d = nc.declare_dram_parameter("hlast", [HT, 128, NB], F32, isOutput=True)

    scr = nc.dram_tensor("lnscr", [NBLK, 2048], F32)

    with tile.TileContext(nc) as tc:
        with (
            tc.tile_pool(name="const", bufs=1) as cst,
            tc.tile_pool(name="sb", bufs=1) as sb,
            tc.tile_pool(name="rp", bufs=4, space="PSUM") as rp,
            tc.tile_pool(name="ip", bufs=4, space="PSUM") as ip,
        ):
            # ---- constants (wih first: needed by the t=0 prefill) ----
            wih_sb, whh_sb, h0m_sb = [], [], []
            for k in range(HT):
                w1 = cst.tile([128, 3 * H], BF16, name=f"wih_sb{k}", tag=f"wih{k}")
                nc.sync.dma_start(out=w1, in_=wih_d[k, :, :])
                wih_sb.append(w1)
            x0 = []
            for k in range(HT):
                x1 = sb.tile([128, S], BF16, name=f"xt0_{k}", tag="xt", bufs=6)
                nc.sync.dma_start(out=x1, in_=xs_d[k, :, 0, :])
                x0.append(x1)
            for k in range(HT):
                w2 = cst.tile([128, 3 * H], BF16, name=f"whh_sb{k}", tag=f"whh{k}")
                nc.sync.dma_start(out=w2, in_=whh_d[k, :, :])
                whh_sb.append(w2)
                hm = cst.tile([128, NB], F32R, name=f"h0m_sb{k}", tag=f"h0m{k}")
                nc.sync.dma_start(out=hm, in_=h0m_d[k, :, :])
                h0m_sb.append(hm)
            brz_sb = cst.tile([128, 8], F32, name="brz_sb", tag="brz")
            nc.sync.dma_start(out=brz_sb, in_=brz_d[:, :])
            bhn_sb = cst.tile([128, HT], F32, name="bhn_sb", tag="bhn")
            nc.sync.dma_start(out=bhn_sb, in_=bhn_d[:, :])
            bin_sb = cst.tile([128, HT], F32, name="bin_sb", tag="bin")
            nc.sync.dma_start(out=bin_sb, in_=bin_d[:, :])
            gam_sb = cst.tile([128, HT], F32, name="gam_sb", tag="gam")
            nc.sync.dma_start(out=gam_sb, in_=gam_d[:, :])
            bet_sb = cst.tile([128, HT], F32, name="bet_sb", tag="bet")
            nc.sync.dma_start(out=bet_sb, in_=bet_d[:, :])
            ones_sb = cst.tile([128, 1], F32R, name="ones_sb", tag="ones")
            nc.sync.dma_start(out=ones_sb, in_=ones_d[:, :])
            eps_sb = cst.tile([128, 1], F32, name="eps_sb", tag="eps")
            nc.vector.memset(eps_sb, 1e-5)

            # ---- initial (zero) state ----
            s_cur = []
            for k in range(HT):
                st = sb.tile([128, S], BF16, name=f"s_init{k}", tag="state", bufs=6)
                nc.vector.memset(st, 0.0)
                s_cur.append(st)

            out_flat = [out_d[k, :, :, :].rearrange("p t s -> p (t s)") for k in range(HT)]

            def load_x(t):
                xt = []
                for k in range(HT):
                    x1 = sb.tile([128, S], BF16, name=f"xt{t}_{k}", tag="xt", bufs=6)
                    nc.sync.dma_start(out=x1, in_=xs_d[k, :, t, :])
                    xt.append(x1)
                return xt

            def load_xr(t):
                # f32 copy of x for the residual path (main steps only)
                xr = []
                for k in range(HT):
                    x1 = sb.tile([128, S], F32, name=f"xr{t}_{k}", tag="xr", bufs=6)
                    nc.sync.dma_start(out=x1, in_=xr_d[k, :, t - R, :])
                    xr.append(x1)
                return xr

            def prefill_gxn(t, xt):
                # complete psum groups for gx_n of step t (wih only),
                # drained straight to SBUF on ACT with b_in folded in
                gx_ps = [
                    ip.tile([128, 512], F32, name=f"gx{t}_{j}", tag="ip")
                    for j in range(2)
                ]
                for k4 in range(4):
                    j = 8 + k4
                    oap = gx_ps[k4 // 2][:, (k4 % 2) * 256 : (k4 % 2) * 256 + 256]
                    for k in range(HT):
                        nc.tensor.matmul(
                            oap, wih_sb[k][:, j * 128 : (j + 1) * 128], xt[k],
                            start=(k == 0), stop=(k == HT - 1))
                gxs = []
                for k in range(HT):
                    g1 = sb.tile([128, S], F32, name=f"gxs{t}_{k}", tag="gxs", bufs=8)
                    nc.scalar.activation(
                        out=g1,
                        in_=gx_ps[k // 2][:, (k % 2) * 256 : (k % 2) * 256 + 256],
                        func=AF.Identity, bias=bin_sb[:, k : k + 1], scale=1.0)
                    gxs.append(g1)
                return gxs

            def prefill_r(t, xt):
                # open accumulation groups for the r gate of step t
                r_ps = [
                    rp.tile([128, 256], F32, name=f"r{t}_{j}", tag="rp")
                    for j in range(4)
                ]
                for j in range(4):
                    for k in range(HT):
                        nc.tensor.matmul(
                            r_ps[j], wih_sb[k][:, j * 128 : (j + 1) * 128], xt[k],
                            start=(k == 0), stop=False)
                return r_ps

            def emit_ln(blk, y_blk, y2):
                """LayerNorm for finished block blk (inputs long-ready)."""
                FB = BLK * S  # 1024
                mu_ps = [
                    ip.tile([1, 512], F32, name=f"mu{blk}_{h}", tag="ip")
                    for h in range(2)
                ]
                ss_ps = [
                    ip.tile([1, 512], F32, name=f"ss{blk}_{h}", tag="ip")
                    for h in range(2)
                ]
                for half in range(2):
                    for k in range(HT):
                        nc.tensor.matmul(
                            mu_ps[half], ones_sb,
                            y_blk[k][:, half * 512 : (half + 1) * 512],
                            start=(k == 0), stop=(k == HT - 1))
                    for k in range(HT):
                        nc.tensor.matmul(
                            ss_ps[half], ones_sb,
                            y2[k][:, half * 512 : (half + 1) * 512],
                            start=(k == 0), stop=(k == HT - 1))
                # drain row sums (scaled by 1/H) into one bounce tile
                bnc = sb.tile([1, 2048], F32, name=f"bnc{blk}", tag="bnc", bufs=1)
                for half in range(2):
                    nc.scalar.activation(
                        out=bnc[:, half * 512 : (half + 1) * 512],
                        in_=mu_ps[half], func=AF.Identity, scale=1.0 / H)
                    nc.scalar.activation(
                        out=bnc[:, 1024 + half * 512 : 1024 + (half + 1) * 512],
                        in_=ss_ps[half], func=AF.Identity, scale=1.0 / H)
                nc.scalar.dma_start(out=scr[blk : blk + 1, :], in_=bnc)
                m1_bc = sb.tile([128, FB], F32, name=f"m1bc{blk}", tag="m1bc", bufs=2)
                s1_bc = sb.tile([128, FB], F32, name=f"s1bc{blk}", tag="s1bc", bufs=2)
                nc.scalar.dma_start(out=m1_bc, in_=_bcast_ap(scr[blk, 0:1024]))
                nc.scalar.dma_start(out=s1_bc, in_=_bcast_ap(scr[blk, 1024:2048]))
                # stats at 128 partitions: var = E[y^2] - mu^2
                q = sb.tile([128, FB], F32, name=f"q{blk}", tag="lnq", bufs=2)
                nc.vector.scalar_tensor_tensor(
                    out=q, in0=m1_bc, scalar=-1.0, in1=m1_bc,
                    op0=ALU.mult, op1=ALU.mult)
                var = sb.tile([128, FB], F32, name=f"var{blk}", tag="lnv", bufs=2)
                nc.gpsimd.tensor_add(var, q, s1_bc)
                std = sb.tile([128, FB], F32, name=f"std{blk}", tag="lns", bufs=2)
                nc.scalar.activation(
                    out=std, in_=var, func=AF.Sqrt, bias=eps_sb, scale=1.0)
                rst = sb.tile([128, FB], F32, name=f"rst{blk}", tag="lnr", bufs=2)
                nc.vector.reciprocal_approx_fast(out=rst, in_=std)
                for k in range(HT):
                    yn = sb.tile([128, FB], F32, name=f"yn{blk}_{k}", tag="yn", bufs=2)
                    nc.vector.scalar_tensor_tensor(
                        out=yn, in0=m1_bc, scalar=-1.0, in1=y_blk[k],
                        op0=ALU.mult, op1=ALU.add)
                    nc.gpsimd.tensor_mul(yn, yn, rst)
                    if not triv_gb:
                        nc.vector.tensor_scalar(
                            out=yn, in0=yn,
                            scalar1=gam_sb[:, k : k + 1],
                            scalar2=bet_sb[:, k : k + 1],
                            op0=ALU.mult, op1=ALU.add)
                    nc.sync.dma_start(
                        out=out_flat[k][:, blk * FB : (blk + 1) * FB], in_=yn)

            xt = x0
            gxs_cur = prefill_gxn(0, xt)
            r_ps = prefill_r(0, xt)

            y_blk = None
            y2 = None
            xr_cur = None
            pend_ln = None  # (blk, y_blk, y2) finished, LN not yet emitted
            for t in range(T):
                main = t >= R
                toff = (t - R) % BLK
                blk = (t - R) // BLK

                if t == R:
                    xr_cur = load_xr(t)
                if t + 1 < T:
                    xt_nxt = load_x(t + 1)
                    if t + 1 > R:
                        xr_nxt = load_xr(t + 1)
                    mk = sb.tile([128, S], F32, name=f"mk{t}", tag="mask", bufs=3)
                    nc.scalar.dma_start(out=mk, in_=_bcast_ap(ms_d[t + 1, :]))

                # -- close r groups with the recurrent part --
                # k-outer: the first matmuls need only s_cur[0], so PE can
                # start as soon as the first state tile is masked
                for k in range(HT):
                    for j in range(4):
                        nc.tensor.matmul(
                            r_ps[j], whh_sb[k][:, j * 128 : (j + 1) * 128], s_cur[k],
                            start=False, stop=(k == HT - 1))
                # -- r sigmoids first in the ACT queue (chain-critical) --
                r_t = []
                for k in range(HT):
                    rt = sb.tile([128, S], F32, name=f"rt{t}_{k}", tag="rt", bufs=4)
                    nc.scalar.activation(
                        out=rt, in_=r_ps[k],
                        func=AF.Sigmoid, bias=brz_sb[:, k : k + 1], scale=1.0)
                    r_t.append(rt)
                # -- gh_n (whh only, complete groups) --
                gh_ps = [
                    ip.tile([128, 512], F32, name=f"gh{t}_{j}", tag="ip")
                    for j in range(2)
                ]
                for k4 in range(4):
                    j = 8 + k4
                    oap = gh_ps[k4 // 2][:, (k4 % 2) * 256 : (k4 % 2) * 256 + 256]
                    for k in range(HT):
                        nc.tensor.matmul(
                            oap, whh_sb[k][:, j * 128 : (j + 1) * 128], s_cur[k],
                            start=(k == 0), stop=(k == HT - 1))
                # -- n gate math (DVE/GPS/ACT), chain-order emission --
                gxs = gxs_cur
                n_t = []
                for k in range(HT):
                    stt = sb.tile([128, S], F32, name=f"st{t}_{k}", tag="stt", bufs=4)
                    nc.vector.scalar_tensor_tensor(
                        out=stt,
                        in0=gh_ps[k // 2][:, (k % 2) * 256 : (k % 2) * 256 + 256],
                        scalar=bhn_sb[:, k : k + 1], in1=r_t[k],
                        op0=ALU.add, op1=ALU.mult)
                    u = sb.tile([128, S], F32, name=f"u{t}_{k}", tag="u", bufs=4)
                    nc.gpsimd.tensor_add(u, stt, gxs[k])
                    nt = sb.tile([128, S], F32, name=f"nt{t}_{k}", tag="nt", bufs=4)
                    nc.scalar.activation(
                        out=nt, in_=u, func=AF.Tanh, scale=1.0)
                    n_t.append(nt)
                # -- z gate (whh + wih complete groups, in-step) --
                z_ps = [
                    ip.tile([128, 512], F32, name=f"z{t}_{j}", tag="ip")
                    for j in range(2)
                ]
                for j4 in range(4):
                    j = 4 + j4
                    oap = z_ps[j4 // 2][:, (j4 % 2) * 256 : (j4 % 2) * 256 + 256]
                    for k in range(HT):
                        nc.tensor.matmul(
                            oap, whh_sb[k][:, j * 128 : (j + 1) * 128], s_cur[k],
                            start=(k == 0), stop=False)
                    for k in range(HT):
                        nc.tensor.matmul(
                            oap, wih_sb[k][:, j * 128 : (j + 1) * 128], xt[k],
                            start=False, stop=(k == HT - 1))
                z_t = []
                for k in range(HT):
                    j = 4 + k
                    zt = sb.tile([128, S], F32, name=f"zt{t}_{k}", tag="zt", bufs=4)
                    nc.scalar.activation(
                        out=zt, in_=z_ps[k // 2][:, (k % 2) * 256 : (k % 2) * 256 + 256],
                        func=AF.Sigmoid, bias=brz_sb[:, j : j + 1], scale=1.0)
                    z_t.append(zt)
                # -- hidden update: hn = (s - n)*z + n --
                hn = []
                for k in range(HT):
                    t1 = sb.tile([128, S], F32, name=f"t1{t}_{k}", tag="t1", bufs=4)
                    nc.gpsimd.tensor_sub(t1, s_cur[k], n_t[k])
                    t2 = sb.tile([128, S], F32, name=f"t2{t}_{k}", tag="t2", bufs=4)
                    nc.vector.tensor_mul(t2, t1, z_t[k])
                    hh = sb.tile([128, S], F32, name=f"hn{t}_{k}", tag="hn", bufs=4)
                    nc.vector.tensor_add(hh, t2, n_t[k])
                    hn.append(hh)

                # -- next state (masked), h0 injection at entry to main --
                if t + 1 < T:
                    s_nxt = []
                    for k in range(HT):
                        sn = sb.tile([128, S], BF16, name=f"s{t + 1}_{k}",
                                     tag="state", bufs=6)
                        nc.vector.tensor_mul(sn, hn[k], mk)
                        s_nxt.append(sn)
                    if t + 1 == R:
                        for k in range(HT):
                            nc.vector.tensor_copy(
                                s_nxt[k][:, 0:S:C], h0m_sb[k])

                # -- residual into LN block buffer --
                if main:
                    if toff == 0:
                        y_blk = [
                            sb.tile([128, BLK * S], F32R, name=f"yb{blk}_{k}",
                                    tag=f"yb{k}", bufs=2)
                            for k in range(HT)
                        ]
                        y2 = [
                            sb.tile([128, BLK * S], F32R, name=f"y2_{blk}_{k}",
                                    tag=f"y2_{k}", bufs=2)
                            for k in range(HT)
                        ]
                    for k in range(HT):
                        ysl = y_blk[k][:, toff * S : (toff + 1) * S]
                        nc.gpsimd.tensor_add(ysl, hn[k], xr_cur[k])
                        nc.gpsimd.tensor_mul(
                            y2[k][:, toff * S : (toff + 1) * S], ysl, ysl)

                # -- final hidden state (chunk C-1 columns) --
                if t == T - 1:
                    for k in range(HT):
                        nc.sync.dma_start(
                            out=hl_d[k, :, :], in_=hn[k][:, C - 1 : S : C])

                # -- prefill next step (keeps PE busy across the boundary) --
                if t + 1 < T:
                    gxs_cur = prefill_gxn(t + 1, xt_nxt)
                    r_ps = prefill_r(t + 1, xt_nxt)
                    s_cur = s_nxt
                    xt = xt_nxt
                    if t + 1 > R:
                        xr_cur = xr_nxt

                # -- delayed LayerNorm for the previously finished block --
                if main and toff == BLK - 1:
                    if pend_ln is not None:
                        emit_ln(*pend_ln)
                    pend_ln = (blk, y_blk, y2)
            emit_ln(*pend_ln)
    nc.compile()
    return nc


def stage_inputs(input, h, is_initial, W_ih, W_hh, b_ih, b_hh, gamma, beta, R):
    """Host-side sharding/staging. Returns per-core input maps."""
    T = R + KS
    x = np.asarray(input, np.float32)
    h0 = np.asarray(h, np.float32)
    ii = np.asarray(is_initial).reshape(N, L)
    W_ih = np.asarray(W_ih, np.float32)
    W_hh = np.asarray(W_hh, np.float32)
    b_ih = np.asarray(b_ih, np.float32)
    b_hh = np.asarray(b_hh, np.float32)
    gamma = np.asarray(gamma, np.float32)
    beta = np.asarray(beta, np.float32)

    mask = 1.0 - ii.astype(np.float32)  # [N, L]

    # l index per (c, t): warm-up reads the R steps before the chunk;
    # chunk 0's warm-up reads l in [KS-R, KS) (discarded garbage).
    l_for = np.empty((C, T), np.int64)
    for c in range(C):
        for t in range(T):
            l = c * KS + (t - R)
            l_for[c, t] = l if l >= 0 else l + KS

    # weight layouts: wih[k, p, g] = W_ih[g, k*128+p]
    wihT = np.ascontiguousarray(
        W_ih.T.reshape(HT, 128, 3 * H)).astype(ml_dtypes.bfloat16)
    whhT = np.ascontiguousarray(
        W_hh.T.reshape(HT, 128, 3 * H)).astype(ml_dtypes.bfloat16)
    brz = (b_ih + b_hh)[: 2 * H].reshape(8, 128).T.copy()        # [128, 8]
    bhn = b_hh[2 * H :].reshape(HT, 128).T.copy()                # [128, 4]
    binn = b_ih[2 * H :].reshape(HT, 128).T.copy()
    gam = gamma.reshape(HT, 128).T.copy()
    bet = beta.reshape(HT, 128).T.copy()
    ones = np.ones((128, 1), np.float32)

    in_maps = []
    for core in range(NCORES):
        n0 = core * NB
        xc = x[n0 : n0 + NB]              # [NB, L, H]
        # xs[k, p, t, s] = x[n, l_for[c, t], k*128+p], s = n*C + c
        xg = xc[:, l_for, :]              # [NB, C, T, H]
        xs_f32 = np.ascontiguousarray(
            xg.transpose(3, 2, 0, 1).reshape(HT, 128, T, S))
        xs = xs_f32.astype(ml_dtypes.bfloat16)
        xres = np.ascontiguousarray(xs_f32[:, :, R:, :])
        mg = mask[n0 : n0 + NB][:, l_for]  # [NB, C, T]
        ms = np.ascontiguousarray(mg.transpose(2, 0, 1).reshape(T, S))
        m0 = mask[n0 : n0 + NB, 0]         # [NB]
        h0m = np.ascontiguousarray(
            (h0[n0 : n0 + NB] * m0[:, None]).T.reshape(HT, 128, NB))
        in_maps.append({
            "xs": xs, "xres": xres, "ms": ms, "h0m": h0m,
            "wih": wihT, "whh": whhT, "brz": brz, "bhn": bhn, "bin": binn,
            "gam": gam, "bet": bet, "ones": ones,
        })
    return in_maps


def required_warmup(is_initial):
    """Max distance from a chunk boundary back to the latest reset."""
    ii = np.asarray(is_initial).reshape(N, L)
    need = 0
    for c in range(1, C):
        start = c * KS
        sub = ii[:, :start]
        for n in range(N):
            nz = np.nonzero(sub[n])[0]
            gap = start - nz[-1] if len(nz) else start
            need = max(need, gap)
    return need


def unstage_outputs(results):
    out = np.empty((N, L, H), np.float32)
    h_last = np.empty((N, H), np.float32)
    for core in range(NCORES):
        n0 = core * NB
        st = results[core]["out_st"]      # [HT, 128, KS, S]
        o = st.reshape(HT, 128, KS, NB, C).transpose(3, 4, 2, 0, 1)
        out[n0 : n0 + NB] = o.reshape(NB, L, H)
        hl = results[core]["hlast"]       # [HT, 128, NB]
        h_last[n0 : n0 + NB] = hl.transpose(2, 0, 1).reshape(NB, H)
    h_exp = np.broadcast_to(h_last[:, None, :], (N, L, H)).copy()
    return out, h_exp


_PROGRAM_CACHE = {}


def kernel(input, h, is_initial, W_ih, W_hh, b_ih, b_hh, gamma, beta):
    need = required_warmup(is_initial)
    R = max(8, min(need, KS))
    triv = bool(
        np.all(np.asarray(gamma) == 1.0) and np.all(np.asarray(beta) == 0.0))
    key = (R, triv)
    if key not in _PROGRAM_CACHE:
        _PROGRAM_CACHE[key] = build_program(R, triv_gb=triv)
    nc = _PROGRAM_CACHE[key]
    in_maps = stage_inputs(
        input, h, is_initial, W_ih, W_hh, b_ih, b_hh, gamma, beta, R)
    res = run_bass_kernel_spmd(nc, in_maps, list(range(NCORES))).results
    return unstage_outputs(res)


# revision 27
# speedup vs baseline: 1.3795x; 1.3795x over previous
"""Trainium2 Bass kernel for masked-GRU + residual + LayerNorm.

Problem: N=128 sequences of length L=512, hidden H=512.
  gx = x @ W_ih.T + b_ih            (precomputable input projection)
  per step l: hc = h * (1-is_initial[l]); gh = hc @ W_hh.T + b_hh
    r = sig(gx_r+gh_r); z = sig(gx_z+gh_z); n = tanh(gx_n + r*gh_n)
    h = (1-z)*n + z*hc
  out = LayerNorm(seq + x) * gamma + beta;  h_exp = broadcast(h_last)

Strategy:
  * Data parallel: 16 batch rows per core (8 cores).
  * Sequence-chunk parallel: 16 chunks of 32 steps as independent
    columns, exact via an R-step warm-up.  R=6 only: the rare columns
    whose latest reset is further back get their entering state computed
    on the host (a few tiny GRU chains) and injected at t=R.
  * All recurrent/projection matmuls in bf16 (weights + moving) ->
    fast weight load; f32 PSUM accumulate.  A separate f32 copy of x
    feeds the residual so output precision keeps margin.
  * Per-step emission keeps ACT/DVE queues chain-first; the gate-update
    chain is split across Vector and Pool per k-tile so the next state
    is ready inside the PE's prefill window (no PE idle -> HAM K=8/8).
  * LayerNorm per 4-step block, delayed one block and spread over its 4
    steps.  Stats are broadcast-reduced on the PE (ones[128,128]
    stationary), so mean/rstd appear on all 128 partitions with no
    DRAM bounce and no 1-partition vector work.
"""
import sys

sys.path.insert(0, "/opt/trn_rl_repo")

import numpy as np
import ml_dtypes

import concourse.bass as bass
import concourse.tile as tile
from concourse import bacc, mybir
from concourse.bass_utils import run_bass_kernel_spmd

F32 = mybir.dt.float32
F32R = mybir.dt.float32r
BF16 = mybir.dt.bfloat16
AF = mybir.ActivationFunctionType
ALU = mybir.AluOpType

N, L, H = 128, 512, 512
NCORES = 8
NB = N // NCORES          # batch rows per core = 16
C = 16                    # chunks per sequence
KS = L // C               # main steps per chunk = 32
S = NB * C                # columns per core = 256
HT = H // 128             # h partition tiles = 4
BLK = 4                   # LN block (main steps)
NBLK = KS // BLK          # 8
RWARM = 6                 # device warm-up steps; longer gaps host-patched


def _bcast_ap(row_ap, parts=128):
    """DRAM row AP -> partition-broadcast AP (step 0 over partitions)."""
    return bass.AP(
        tensor=row_ap.tensor,
        offset=row_ap.offset,
        ap=[[0, parts]] + [list(d) for d in row_ap.ap],
    )


def build_program(R=RWARM, triv_gb=False):
    T = R + KS
    nc = bacc.Bacc("TRN2", target_bir_lowering=False)

    xs_d = nc.declare_dram_parameter("xs", [HT, 128, T, S], BF16, isOutput=False)
    xr_d = nc.declare_dram_parameter("xres", [HT, 128, KS, S], F32, isOutput=False)
    ms_d = nc.declare_dram_parameter("ms", [T, S], F32, isOutput=False)
    kv_d = nc.declare_dram_parameter("kv", [HT + 1, 128, S], BF16, isOutput=False)
    wih_d = nc.declare_dram_parameter("wih", [HT, 128, 3 * H], BF16, isOutput=False)
    whh_d = nc.declare_dram_parameter("whh", [HT, 128, 3 * H], BF16, isOutput=False)
    brz_d = nc.declare_dram_parameter("brz", [128, 8], F32, isOutput=False)
    bhn_d = nc.declare_dram_parameter("bhn", [128, HT], F32, isOutput=False)
    bin_d = nc.declare_dram_parameter("bin", [128, HT], F32, isOutput=False)
    gam_d = nc.declare_dram_parameter("gam", [128, HT], F32, isOutput=False)
    bet_d = nc.declare_dram_parameter("bet", [128, HT], F32, isOutput=False)
    on128_d = nc.declare_dram_parameter("on128", [128, 128], F32R, isOutput=False)

    out_d = nc.declare_dram_parameter("out_st", [HT, 128, KS, S], F32, isOutput=True)
    hl_d = nc.declare_dram_parameter("hlast", [HT, 128, NB], F32, isOutput=True)

    with tile.TileContext(nc) as tc:
        with (
            tc.tile_pool(name="const", bufs=1) as cst,
            tc.tile_pool(name="sb", bufs=1) as sb,
            tc.tile_pool(name="rp", bufs=4, space="PSUM") as rp,
            tc.tile_pool(name="ip", bufs=4, space="PSUM") as ip,
        ):
            # ---- constants (wih first: needed by the t=0 prefill) ----
            wih_sb, whh_sb = [], []
            for k in range(HT):
                w1 = cst.tile([128, 3 * H], BF16, name=f"wih_sb{k}", tag=f"wih{k}")
                nc.sync.dma_start(out=w1, in_=wih_d[k, :, :])
                wih_sb.append(w1)
            x0 = []
            for k in range(HT):
                x1 = sb.tile([128, S], BF16, name=f"xt0_{k}", tag="xt", bufs=6)
                nc.sync.dma_start(out=x1, in_=xs_d[k, :, 0, :])
                x0.append(x1)
            for k in range(HT):
                w2 = cst.tile([128, 3 * H], BF16, name=f"whh_sb{k}", tag=f"whh{k}")
                nc.sync.dma_start(out=w2, in_=whh_d[k, :, :])
                whh_sb.append(w2)
            keep_sb = cst.tile([128, S], BF16, name="keep_sb", tag="keep")
            nc.sync.dma_start(out=keep_sb, in_=kv_d[HT, :, :])
            inj_sb = []
            for k in range(HT):
                iv = cst.tile([128, S], BF16, name=f"inj_sb{k}", tag=f"inj{k}")
                nc.sync.dma_start(out=iv, in_=kv_d[k, :, :])
                inj_sb.append(iv)
            brz_sb = cst.tile([128, 8], F32, name="brz_sb", tag="brz")
            nc.sync.dma_start(out=brz_sb, in_=brz_d[:, :])
            bhn_sb = cst.tile([128, HT], F32, name="bhn_sb", tag="bhn")
            nc.sync.dma_start(out=bhn_sb, in_=bhn_d[:, :])
            bin_sb = cst.tile([128, HT], F32, name="bin_sb", tag="bin")
            nc.sync.dma_start(out=bin_sb, in_=bin_d[:, :])
            gam_sb = cst.tile([128, HT], F32, name="gam_sb", tag="gam")
            nc.sync.dma_start(out=gam_sb, in_=gam_d[:, :])
            bet_sb = cst.tile([128, HT], F32, name="bet_sb", tag="bet")
            nc.sync.dma_start(out=bet_sb, in_=bet_d[:, :])
            on128_sb = cst.tile([128, 128], F32R, name="on128_sb", tag="on128")
            nc.sync.dma_start(out=on128_sb, in_=on128_d[:, :])
            eps_sb = cst.tile([128, 1], F32, name="eps_sb", tag="eps")
            nc.vector.memset(eps_sb, 1e-5)

            # ---- initial (zero) state ----
            s_cur = []
            for k in range(HT):
                st = sb.tile([128, S], BF16, name=f"s_init{k}", tag="state", bufs=6)
                nc.vector.memset(st, 0.0)
                s_cur.append(st)

            out_flat = [out_d[k, :, :, :].rearrange("p t s -> p (t s)") for k in range(HT)]

            def load_x(t):
                xt = []
                for k in range(HT):
                    x1 = sb.tile([128, S], BF16, name=f"xt{t}_{k}", tag="xt", bufs=6)
                    nc.sync.dma_start(out=x1, in_=xs_d[k, :, t, :])
                    xt.append(x1)
                return xt

            def load_xr(t):
                # f32 copy of x for the residual path (main steps only)
                xr = []
                for k in range(HT):
                    x1 = sb.tile([128, S], F32, name=f"xr{t}_{k}", tag="xr", bufs=6)
                    nc.sync.dma_start(out=x1, in_=xr_d[k, :, t - R, :])
                    xr.append(x1)
                return xr

            def prefill_gxn(t, xt):
                # complete psum groups for gx_n of step t (wih only),
                # drained straight to SBUF on ACT with b_in folded in
                gx_ps = [
                    ip.tile([128, 512], F32, name=f"gx{t}_{j}", tag="ip")
                    for j in range(2)
                ]
                for k4 in range(4):
                    j = 8 + k4
                    oap = gx_ps[k4 // 2][:, (k4 % 2) * 256 : (k4 % 2) * 256 + 256]
                    for k in range(HT):
                        nc.tensor.matmul(
                            oap, wih_sb[k][:, j * 128 : (j + 1) * 128], xt[k],
                            start=(k == 0), stop=(k == HT - 1))
                gxs = []
                for k in range(HT):
                    g1 = sb.tile([128, S], F32, name=f"gxs{t}_{k}", tag="gxs", bufs=8)
                    nc.scalar.activation(
                        out=g1,
                        in_=gx_ps[k // 2][:, (k % 2) * 256 : (k % 2) * 256 + 256],
                        func=AF.Identity, bias=bin_sb[:, k : k + 1], scale=1.0)
                    gxs.append(g1)
                return gxs

            def prefill_r(t, xt):
                # open accumulation groups for the r gate of step t
                r_ps = [
                    rp.tile([128, 256], F32, name=f"r{t}_{j}", tag="rp")
                    for j in range(4)
                ]
                for j in range(4):
                    for k in range(HT):
                        nc.tensor.matmul(
                            r_ps[j], wih_sb[k][:, j * 128 : (j + 1) * 128], xt[k],
                            start=(k == 0), stop=False)
                return r_ps

            # ---- delayed LayerNorm pieces (block b spread over next block) --
            def ln_a(st):
                """PE broadcast-reduction of sums: mu/ss as [128,512] halves."""
                blk, y_blk, y2 = st["blk"], st["y"], st["y2"]
                mu_ps, ss_ps = [], []
                for half in range(2):
                    m = ip.tile([128, 512], F32, name=f"mu{blk}_{half}", tag="ip")
                    for k in range(HT):
                        nc.tensor.matmul(
                            m, on128_sb, y_blk[k][:, half * 512 : (half + 1) * 512],
                            start=(k == 0), stop=(k == HT - 1))
                    mu_ps.append(m)
                for half in range(2):
                    sq = ip.tile([128, 512], F32, name=f"ss{blk}_{half}", tag="ip")
                    for k in range(HT):
                        nc.tensor.matmul(
                            sq, on128_sb, y2[k][:, half * 512 : (half + 1) * 512],
                            start=(k == 0), stop=(k == HT - 1))
                    ss_ps.append(sq)
                st["mu_ps"], st["ss_ps"] = mu_ps, ss_ps

            def ln_b_dve(st):
                """Early-DVE piece: free the PSUM stat tiles fast."""
                blk, mu_ps, ss_ps = st["blk"], st["mu_ps"], st["ss_ps"]
                FB = BLK * S
                m1n = sb.tile([128, FB], F32, name=f"m1n{blk}", tag="m1n", bufs=2)
                q = sb.tile([128, FB], F32, name=f"q{blk}", tag="lnq", bufs=2)
                for half in range(2):
                    sl = slice(half * 512, (half + 1) * 512)
                    nc.vector.tensor_scalar_mul(m1n[:, sl], mu_ps[half], -1.0 / H)
                    nc.vector.tensor_mul(q[:, sl], m1n[:, sl], mu_ps[half])
                st["m1n"], st["q"] = m1n, q

            def ln_b_rest(st):
                blk, q, ss_ps = st["blk"], st["q"], st["ss_ps"]
                FB = BLK * S
                vH = sb.tile([128, FB], F32, name=f"vH{blk}", tag="lnv", bufs=2)
                for half in range(2):
                    sl = slice(half * 512, (half + 1) * 512)
                    nc.vector.tensor_add(vH[:, sl], q[:, sl], ss_ps[half])
                std = sb.tile([128, FB], F32, name=f"std{blk}", tag="lns", bufs=2)
                nc.scalar.activation(
                    out=std, in_=vH, func=AF.Sqrt, bias=eps_sb, scale=1.0 / H)
                rst = sb.tile([128, FB], F32, name=f"rst{blk}", tag="lnr", bufs=2)
                nc.vector.reciprocal_approx_fast(out=rst, in_=std)
                st["rst"] = rst

            def ln_cd(st, ks):
                blk, y_blk, m1n, rst = st["blk"], st["y"], st["m1n"], st["rst"]
                FB = BLK * S
                for k in ks:
                    yn = sb.tile([128, FB], F32, name=f"yn{blk}_{k}", tag="yn", bufs=4)
                    eng = nc.vector if k % 2 == 0 else nc.gpsimd
                    eng.tensor_add(yn, y_blk[k], m1n)
                    eng.tensor_mul(yn, yn, rst)
                    if not triv_gb:
                        nc.vector.tensor_scalar(
                            out=yn, in0=yn,
                            scalar1=gam_sb[:, k : k + 1],
                            scalar2=bet_sb[:, k : k + 1],
                            op0=ALU.mult, op1=ALU.add)
                    nc.sync.dma_start(
                        out=out_flat[k][:, blk * FB : (blk + 1) * FB], in_=yn)

            xt = x0
            gxs_cur = prefill_gxn(0, xt)
            r_ps = prefill_r(0, xt)

            y_blk = None
            y2 = None
            xr_cur = None
            pend = None  # finished block's LN state dict
            for t in range(T):
                main = t >= R
                toff = (t - R) % BLK
                blk = (t - R) // BLK

                if t == R:
                    xr_cur = load_xr(t)
                if t + 1 < T:
                    xt_nxt = load_x(t + 1)
                    if t + 1 > R:
                        xr_nxt = load_xr(t + 1)
                    mk = sb.tile([128, S], F32, name=f"mk{t}", tag="mask", bufs=3)
                    nc.scalar.dma_start(out=mk, in_=_bcast_ap(ms_d[t + 1, :]))

                # -- early-DVE LN piece (frees PSUM stat tiles quickly) --
                if pend is not None and toff == 1:
                    ln_b_dve(pend)

                # -- close r groups with the recurrent part --
                # k-outer: the first matmuls need only s_cur[0], so PE can
                # start as soon as the first state tile is masked
                for k in range(HT):
                    for j in range(4):
                        nc.tensor.matmul(
                            r_ps[j], whh_sb[k][:, j * 128 : (j + 1) * 128], s_cur[k],
                            start=False, stop=(k == HT - 1))
                # -- r sigmoids first in the ACT queue (chain-critical) --
                r_t = []
                for k in range(HT):
                    rt = sb.tile([128, S], F32, name=f"rt{t}_{k}", tag="rt", bufs=4)
                    nc.scalar.activation(
                        out=rt, in_=r_ps[k],
                        func=AF.Sigmoid, bias=brz_sb[:, k : k + 1], scale=1.0)
                    r_t.append(rt)
                # -- remaining LN-B after the chain-head ACT ops --
                if pend is not None and toff == 1:
                    ln_b_rest(pend)
                # -- gh_n (whh only, complete groups) --
                gh_ps = [
                    ip.tile([128, 512], F32, name=f"gh{t}_{j}", tag="ip")
                    for j in range(2)
                ]
                for k4 in range(4):
                    j = 8 + k4
                    oap = gh_ps[k4 // 2][:, (k4 % 2) * 256 : (k4 % 2) * 256 + 256]
                    for k in range(HT):
                        nc.tensor.matmul(
                            oap, whh_sb[k][:, j * 128 : (j + 1) * 128], s_cur[k],
                            start=(k == 0), stop=(k == HT - 1))
                # -- n gate math, chain-order emission --
                gxs = gxs_cur
                n_t = []
                for k in range(HT):
                    stt = sb.tile([128, S], F32, name=f"st{t}_{k}", tag="stt", bufs=4)
                    nc.vector.scalar_tensor_tensor(
                        out=stt,
                        in0=gh_ps[k // 2][:, (k % 2) * 256 : (k % 2) * 256 + 256],
                        scalar=bhn_sb[:, k : k + 1], in1=r_t[k],
                        op0=ALU.add, op1=ALU.mult)
                    u = sb.tile([128, S], F32, name=f"u{t}_{k}", tag="u", bufs=4)
                    nc.gpsimd.tensor_add(u, stt, gxs[k])
                    nt = sb.tile([128, S], F32, name=f"nt{t}_{k}", tag="nt", bufs=4)
                    nc.scalar.activation(
                        out=nt, in_=u, func=AF.Tanh, scale=1.0)
                    n_t.append(nt)
                # -- z gate (whh + wih complete groups, in-step) --
                z_ps = [
                    ip.tile([128, 512], F32, name=f"z{t}_{j}", tag="ip")
                    for j in range(2)
                ]
                for j4 in range(4):
                    j = 4 + j4
                    oap = z_ps[j4 // 2][:, (j4 % 2) * 256 : (j4 % 2) * 256 + 256]
                    for k in range(HT):
                        nc.tensor.matmul(
                            oap, whh_sb[k][:, j * 128 : (j + 1) * 128], s_cur[k],
                            start=(k == 0), stop=False)
                    for k in range(HT):
                        nc.tensor.matmul(
                            oap, wih_sb[k][:, j * 128 : (j + 1) * 128], xt[k],
                            start=False, stop=(k == HT - 1))
                z_t = []
                for k in range(HT):
                    j = 4 + k
                    zt = sb.tile([128, S], F32, name=f"zt{t}_{k}", tag="zt", bufs=4)
                    nc.scalar.activation(
                        out=zt, in_=z_ps[k // 2][:, (k % 2) * 256 : (k % 2) * 256 + 256],
                        func=AF.Sigmoid, bias=brz_sb[:, j : j + 1], scale=1.0)
                    z_t.append(zt)
                # -- hidden update chain, split DVE/Pool by k --
                hn = []
                s_nxt = [None] * HT if t + 1 < T else None
                for k in range(HT):
                    eng = nc.vector if k % 2 == 0 else nc.gpsimd
                    t1 = sb.tile([128, S], F32, name=f"t1{t}_{k}", tag="t1", bufs=4)
                    eng.tensor_sub(t1, s_cur[k], n_t[k])
                    t2 = sb.tile([128, S], F32, name=f"t2{t}_{k}", tag="t2", bufs=4)
                    eng.tensor_mul(t2, t1, z_t[k])
                    hh = sb.tile([128, S], F32, name=f"hn{t}_{k}", tag="hn", bufs=4)
                    eng.tensor_add(hh, t2, n_t[k])
                    hn.append(hh)
                    if t + 1 < T:
                        sn = sb.tile([128, S], BF16, name=f"s{t + 1}_{k}",
                                     tag="state", bufs=6)
                        nc.vector.tensor_mul(sn, hh, mk)
                        s_nxt[k] = sn

                # -- state injection at entry to main (h0 + host patches) --
                if t + 1 == R:
                    for k in range(HT):
                        nc.vector.tensor_mul(s_nxt[k], s_nxt[k], keep_sb)
                        nc.vector.tensor_add(s_nxt[k], s_nxt[k], inj_sb[k])

                # -- residual into LN block buffer --
                if main:
                    if toff == 0:
                        y_blk = [
                            sb.tile([128, BLK * S], F32R, name=f"yb{blk}_{k}",
                                    tag=f"yb{k}", bufs=2)
                            for k in range(HT)
                        ]
                        y2 = [
                            sb.tile([128, BLK * S], F32R, name=f"y2_{blk}_{k}",
                                    tag=f"y2_{k}", bufs=2)
                            for k in range(HT)
                        ]
                    for k in range(HT):
                        ysl = y_blk[k][:, toff * S : (toff + 1) * S]
                        nc.gpsimd.tensor_add(ysl, hn[k], xr_cur[k])
                        nc.gpsimd.tensor_mul(
                            y2[k][:, toff * S : (toff + 1) * S], ysl, ysl)

                # -- final hidden state (chunk C-1 columns) --
                if t == T - 1:
                    for k in range(HT):
                        nc.sync.dma_start(
                            out=hl_d[k, :, :], in_=hn[k][:, C - 1 : S : C])

                # -- prefill next step (keeps PE busy across the boundary) --
                if t + 1 < T:
                    gxs_cur = prefill_gxn(t + 1, xt_nxt)
                    r_ps = prefill_r(t + 1, xt_nxt)
                    s_cur = s_nxt
                    xt = xt_nxt
                    if t + 1 > R:
                        xr_cur = xr_nxt

                # -- spread LN pieces of the previously finished block --
                if main and pend is not None:
                    if toff == 0:
                        ln_a(pend)
                    elif toff == 2:
                        ln_cd(pend, (0, 1))
                    elif toff == 3:
                        ln_cd(pend, (2, 3))
                        pend = None
                if main and toff == BLK - 1:
                    pend = {"blk": blk, "y": y_blk, "y2": y2}
            # tail: LN of the final block
            ln_a(pend)
            ln_b_dve(pend)
            ln_b_rest(pend)
            ln_cd(pend, (0, 1, 2, 3))
    nc.compile()
    return nc


def _gru_patch(x_row, mask_row, W_ihT, W_hhT, b_ih, b_hh, l0, l1):
    """h after steps l0..l1 for one row, entering state 0 (mask[l0]==0)."""
    hc = np.zeros(H, np.float32)
    for l in range(l0, l1 + 1):
        hcm = hc * mask_row[l]
        gx = x_row[l] @ W_ihT + b_ih
        gh = hcm @ W_hhT + b_hh
        r = 1.0 / (1.0 + np.exp(-(gx[:H] + gh[:H])))
        z = 1.0 / (1.0 + np.exp(-(gx[H : 2 * H] + gh[H : 2 * H])))
        nn = np.tanh(gx[2 * H :] + r * gh[2 * H :])
        hc = (1.0 - z) * nn + z * hcm
    return hc


def stage_inputs(input, h, is_initial, W_ih, W_hh, b_ih, b_hh, gamma, beta, R):
    """Host-side sharding/staging. Returns per-core input maps."""
    T = R + KS
    x = np.asarray(input, np.float32)
    h0 = np.asarray(h, np.float32)
    ii = np.asarray(is_initial).reshape(N, L)
    W_ih = np.asarray(W_ih, np.float32)
    W_hh = np.asarray(W_hh, np.float32)
    b_ih = np.asarray(b_ih, np.float32)
    b_hh = np.asarray(b_hh, np.float32)
    gamma = np.asarray(gamma, np.float32)
    beta = np.asarray(beta, np.float32)

    mask = 1.0 - ii.astype(np.float32)  # [N, L]

    # l index per (c, t): warm-up reads the R steps before the chunk;
    # chunk 0's warm-up reads l in [KS-R, KS) (discarded garbage).
    l_for = np.empty((C, T), np.int64)
    for c in range(C):
        for t in range(T):
            l = c * KS + (t - R)
            l_for[c, t] = l if l >= 0 else l + KS

    # host patches: entering state for (n, c) whose latest reset precedes
    # the chunk start by more than R steps
    W_ihT, W_hhT = W_ih.T.copy(), W_hh.T.copy()
    inj = np.zeros((N, C, H), np.float32)     # value injected at t=R
    keep = np.ones((N, C), np.float32)        # 1 -> keep warm-up state
    keep[:, 0] = 0.0
    inj[:, 0, :] = h0 * mask[:, 0:1]
    for c in range(1, C):
        start = c * KS
        for n in range(N):
            nz = np.nonzero(ii[n, :start])[0]
            gap = start - nz[-1] if len(nz) else start
            if gap > R:
                l0 = nz[-1] if len(nz) else 0
                hp = _gru_patch(x[n], mask[n], W_ihT, W_hhT, b_ih, b_hh,
                                l0, start - 1)
                inj[n, c, :] = hp * mask[n, start]
                keep[n, c] = 0.0

    # weight layouts: wih[k, p, g] = W_ih[g, k*128+p]
    wihT = np.ascontiguousarray(
        W_ih.T.reshape(HT, 128, 3 * H)).astype(ml_dtypes.bfloat16)
    whhT = np.ascontiguousarray(
        W_hh.T.reshape(HT, 128, 3 * H)).astype(ml_dtypes.bfloat16)
    brz = (b_ih + b_hh)[: 2 * H].reshape(8, 128).T.copy()        # [128, 8]
    bhn = b_hh[2 * H :].reshape(HT, 128).T.copy()                # [128, 4]
    binn = b_ih[2 * H :].reshape(HT, 128).T.copy()
    gam = gamma.reshape(HT, 128).T.copy()
    bet = beta.reshape(HT, 128).T.copy()
    on128 = np.ones((128, 128), np.float32)

    in_maps = []
    for core in range(NCORES):
        n0 = core * NB
        xc = x[n0 : n0 + NB]              # [NB, L, H]
        # xs[k, p, t, s] = x[n, l_for[c, t], k*128+p], s = n*C + c
        xg = xc[:, l_for, :]              # [NB, C, T, H]
        xs_f32 = np.ascontiguousarray(
            xg.transpose(3, 2, 0, 1).reshape(HT, 128, T, S))
        xs = xs_f32.astype(ml_dtypes.bfloat16)
        xres = np.ascontiguousarray(xs_f32[:, :, R:, :])
        mg = mask[n0 : n0 + NB][:, l_for]  # [NB, C, T]
        ms = np.ascontiguousarray(mg.transpose(2, 0, 1).reshape(T, S))
        # kv[0:HT] = injected values, kv[HT] = keep mask (bcast over parts)
        kv = np.empty((HT + 1, 128, S), np.float32)
        kv[:HT] = inj[n0 : n0 + NB].transpose(2, 0, 1).reshape(HT, 128, S)
        kv[HT] = np.broadcast_to(
            keep[n0 : n0 + NB].reshape(1, S), (128, S))
        in_maps.append({
            "xs": xs, "xres": xres, "ms": ms,
            "kv": kv.astype(ml_dtypes.bfloat16),
            "wih": wihT, "whh": whhT, "brz": brz, "bhn": bhn, "bin": binn,
            "gam": gam, "bet": bet, "on128": on128,
        })
    return in_maps


def unstage_outputs(results):
    out = np.empty((N, L, H), np.float32)
    h_last = np.empty((N, H), np.float32)
    for core in range(NCORES):
        n0 = core * NB
        st = results[core]["out_st"]      # [HT, 128, KS, S]
        o = st.reshape(HT, 128, KS, NB, C).transpose(3, 4, 2, 0, 1)
        out[n0 : n0 + NB] = o.reshape(NB, L, H)
        hl = results[core]["hlast"]       # [HT, 128, NB]
        h_last[n0 : n0 + NB] = hl.transpose(2, 0, 1).reshape(NB, H)
    h_exp = np.broadcast_to(h_last[:, None, :], (N, L, H)).copy()
    return out, h_exp


_PROGRAM_CACHE = {}


def kernel(input, h, is_initial, W_ih, W_hh, b_ih, b_hh, gamma, beta):
    R = RWARM
    triv = bool(
        np.all(np.asarray(gamma) == 1.0) and np.all(np.asarray(beta) == 0.0))
    key = (R, triv)
    if key not in _PROGRAM_CACHE:
        _PROGRAM_CACHE[key] = build_program(R, triv_gb=triv)
    nc = _PROGRAM_CACHE[key]
    in_maps = stage_inputs(
        input, h, is_initial, W_ih, W_hh, b_ih, b_hh, gamma, beta, R)
    res = run_bass_kernel_spmd(nc, in_maps, list(range(NCORES))).results
    return unstage_outputs(res)
